# revision 25
# baseline (speedup 1.0000x reference)
"""Trainium2 Bass kernel for nn_BasicBlock (binarized ResNet basic block).

Computation (see problem reference):
    residual = x
    out = psum_conv3x3(sign(x), sign(w1))        # 3x3 'same' conv, saturating acc
    out = bn1(out); out = hardtanh(out)
    out = psum_conv3x3(sign(out), sign(w2))
    out = bn2(out); out = out + residual; out = hardtanh(out)

Key facts exploited:
  * C=128 channels = one GROUP, 9 taps of |partial| <= 128 each, so the
    running accumulator magnitude is <= 9*128 = 1152 < THRESH=8000: the
    saturation clip NEVER binds. The conv is a plain 3x3 conv over sign
    values, all arithmetic exact small integers -> freely reorderable and
    exactly representable in fp8e4/bf16 inputs with fp32 PSUM accumulation.
  * sign(hardtanh(v)) == sign(v), so the first hardtanh can be folded into
    the sign feeding conv2.
  * Each conv = 9 shifted-window taps (K=C=128 on partitions) into one PSUM
    accumulation group over a zero-padded row-stride-64 fp8 sign image:
    4 fp8 DoubleRow matmuls (vertically adjacent tap pairs at +RW, plus the
    (r2,c0)+(r2,c1) pair via a col-shifted copy at +SHIFT) and 1 normal
    fp8 matmul for the odd tap (r2,c2).
  * Host-side prep: sign(x) is computed on the host and shipped as fp8;
    the residual arrives as x+b2 in bf16 (bias of the second BN folded
    in). BOTH col-shifted twins are derived on-chip with 1-byte-offset
    copies (GpSimd for sign(x), Scalar for the conv2 image right behind
    its eviction ACT), halving sign-image HBM traffic.
  * Every [128, n] DMA costs ~2.2us of shared DMA-backend time regardless
    of n (the per-partition descriptor walk dominates), so steady-state
    inputs move as ONE whole-image transfer per ring per image (sync:
    main sign image + the y store, scalar: residual); only image 0 is
    split into four pipelined row pieces, with w1 fronting the scalar
    ring so the first LDWEIGHTS fires ~10us in.
  * y is returned as bf16 (quantization ~2^-9 against a 2e-2 budget).

Sharding: data-parallel over batch: 64 images -> 8 cores x 8 images.
"""

import sys

sys.path.insert(0, "/opt/trn_rl_repo")

import numpy as np
import ml_dtypes

import concourse.bass as bass
import concourse.bacc as bacc
import concourse.mybir as mybir
import concourse.tile as tile
from concourse.bass_utils import run_bass_kernel_spmd

# ---------------------------------------------------------------- constants

N_CORES = 8
B, C, H, W = 64, 128, 56, 56
BL = B // N_CORES            # images per core
HP = H + 2                   # padded rows
RW = 64                      # padded row width (stride): 56 valid + pads,
                             # 64 so the DoubleRow plane step (+RW) is 16-aligned
CHUNK_ROWS = 8               # output rows per PSUM chunk
NFLAT = CHUNK_ROWS * RW      # 512 flat psum columns per chunk (one bank)
N_CHUNKS = H // CHUNK_ROWS   # 7
EPS = 1e-5
SHIFT = HP * RW              # offset of the col-shifted copy inside xs/ts
WCOLS = 2 * (4 * 256 + 128)  # fp8 weight table columns (2 convs x 1152)

F32 = mybir.dt.float32
BF16 = mybir.dt.bfloat16
FP8 = mybir.dt.float8e4

# number of PE p-state warmup matmuls burned while the first input DMAs land
# (sized to end ~10.3us, just as w1 + the first sign piece post: earlier
# counts left a pre-stream idle that sagged the PE p-state)
WARMUP_MM = 17

_NC_CACHE = None


def _build_nc():
    """Build the per-core Bass module (same NEFF on all 8 cores)."""
    nc = bacc.Bacc("TRN2", debug=False)

    # host-binarized sign(x) in fp8 in the zero-padded row-stride-64 layout
    # (the col-shifted twin is derived on-chip), and the b2-biased residual
    s_d = nc.dram_tensor("s", [BL, C, 2 * SHIFT], FP8, kind="ExternalInput").ap()
    xr_d = nc.dram_tensor("xr", [BL, C, H, W], BF16, kind="ExternalInput").ap()
    # host-prepped fp8 weight tables, per conv: 3 DoubleRow pair tables
    # [cin, 2*cout] for (r0,r1) at c=0,1,2 then the (r2,c0)+(r2,c1) pair and
    # the plain (r2,c2) table
    w_d = nc.dram_tensor("w", [C, WCOLS], FP8, kind="ExternalInput").ap()
    # folded BN params per channel: [:,0]=inv1 [:,1]=b1 [:,2]=inv2
    bn_d = nc.dram_tensor("bn", [C, 4], F32, kind="ExternalInput").ap()
    y_d = nc.dram_tensor("y", [BL, C, H, W], BF16, kind="ExternalOutput").ap()

    SIGN = mybir.ActivationFunctionType.Sign
    DR = mybir.MatmulPerfMode.DoubleRow
    MIN, MAX = mybir.AluOpType.min, mybir.AluOpType.max

    # image-0 main sign-image DMA pieces (padded rows); chunk k reads rows
    # 8k..8k+9, split so each piece lands just before the PE needs it
    PIECES0 = ((0, 10), (10, 16), (26, 16), (42, 16))
    # on-chip shift-copy pieces: chunk k's step-4 reads shifted rows
    # 8k+2..8k+9 (cols 0..55 only), so piece k covers exactly those rows
    SPIECES = tuple((8 * k + 2, 8) for k in range(7))
    # merged variant for steady-state images (one whole-main DMA dep)
    SPIECES3 = ((2, 16), (18, 20), (38, 20))

    with tile.TileContext(nc) as tc:
        with (
            tc.tile_pool(name="sb", bufs=1) as sb,
            tc.tile_pool(name="psum", bufs=4, space="PSUM") as pspool,
        ):
            # junk tiles for PE warmup + ACT table preload; memset on gpsimd
            # (its user stream opens ~1us before vector's) so warmup matmuls
            # can start right as the preamble ends
            junk2 = sb.tile([C, 512], FP8, name="junk2")
            nc.gpsimd.memset(junk2[:, 0:288], 0.0)
            junk = sb.tile([C, 2], F32, name="junk")
            nc.gpsimd.memset(junk[:], 0.0)

            w_sb = sb.tile([C, WCOLS], FP8, name="wsb")
            bn_sb = sb.tile([C, 4], F32, name="bnsb")
            xs_t = [sb.tile([C, 2 * SHIFT], FP8, name=f"xs{j}") for j in range(3)]
            ts_t = [sb.tile([C, 2 * SHIFT], FP8, name=f"ts{j}") for j in range(3)]
            # ring depth 4: the write-after-read dependency of image i's
            # residual load then lands on image i-4's (long finished) final
            # eviction, so the issue never blocks its queue
            xr_t = [sb.tile([C, H, W], BF16, name=f"xr{j}") for j in range(4)]
            o_t = [sb.tile([C, H, W], BF16, name=f"o{j}") for j in range(3)]

            def xs3v(buf):
                return buf[:, 0:SHIFT].rearrange("p (h w) -> p h w", w=RW)

            def zero_pads(eng, buf):
                b3 = xs3v(buf)
                eng.memset(b3[:, 0, :], 0.0)
                eng.memset(b3[:, HP - 1, :], 0.0)
                eng.memset(b3[:, 1 : HP - 1, 0:1], 0.0)
                eng.memset(b3[:, 1 : HP - 1, W + 1 : RW], 0.0)
                # last padded row of the shifted copy is all pad-derived
                eng.memset(buf[:, SHIFT + (HP - 1) * RW : 2 * SHIFT], 0.0)

            # -- startup DMA plan. Every [128,n] transfer costs ~2.2us of
            # the shared DMA backend, so only what gates the first matmuls
            # goes first: image-0 main pieces on sync, conv1 weights + bn +
            # residual on scalar, conv2 weights on gpsimd.
            for r0, nr in PIECES0:
                a, b = r0 * RW, (r0 + nr) * RW
                nc.sync.dma_start(xs_t[0][:, a:b], s_d[0, :, a:b])
            nc.scalar.dma_start(w_sb[:, 0:1152], w_d[:, 0:1152])
            nc.scalar.dma_start(bn_sb[:], bn_d[:])
            nc.scalar.activation(junk[:, 1:2], junk[:, 0:1], SIGN)
            # conv2 weights ride sync BEHIND the image-0 pieces (needed
            # only ~17us in), and the image-0 residual is issued mid-conv1
            # (needed by evict2 ~18us in): keeping both out of the startup
            # burst lets w1 + the first sign piece — the only transfers
            # that gate the first matmul — finish as early as the shared
            # DMA backend allows even on clock-throttled runs
            nc.sync.dma_start(w_sb[:, 1152:WCOLS], w_d[:, 1152:WCOLS])

            def shift_aps(buf, row0, nrows):
                src = bass.AP(
                    tensor=buf.tensor,
                    offset=buf.offset + row0 * RW + 1,
                    ap=[buf.ap[0], [1, nrows * RW]],
                )
                dst = bass.AP(
                    tensor=buf.tensor,
                    offset=buf.offset + SHIFT + row0 * RW,
                    ap=[buf.ap[0], [1, nrows * RW]],
                )
                return dst, src

            def shift_scalar(buf, row0, nrows):
                """shifted[h, w] = main[h, w+1] for rows [row0, row0+nrows)
                on the Scalar engine right behind the eviction ACT."""
                dst, src = shift_aps(buf, row0, nrows)
                nc.scalar.copy(dst, src)

            def shift_copy(eng, buf, piece):
                """One on-chip shift piece: shifted[h, w] = main[h, w+1]
                for rows [r0, r0+nr), cols 0..62 — 2D so the read never
                crosses a row boundary (col 63 of the twin is never read
                by the matmuls, whose taps only touch cols 0..55)."""
                r0, nr = piece
                src = bass.AP(
                    tensor=buf.tensor,
                    offset=buf.offset + r0 * RW + 1,
                    ap=[buf.ap[0], [RW, nr], [1, RW - 1]],
                )
                dst = bass.AP(
                    tensor=buf.tensor,
                    offset=buf.offset + SHIFT + r0 * RW,
                    ap=[buf.ap[0], [RW, nr], [1, RW - 1]],
                )
                eng.tensor_copy(dst, src)

            STEPS = (3, 0, 1, 2, 4)  # single first, shift-pair last: the
            # two non-DR<->DR perf-mode transitions of a chunk sit at its
            # edges, so interleaved pairs share them

            def chunk_matmul(ps, src, conv_idx, h0, step, nrows=CHUNK_ROWS):
                """Emit matmul `step` (0..4) of one output chunk: 4 DoubleRow
                + 1 normal fp8 matmul.

                Steps 0..2 pair the vertically adjacent taps (r0,c)+(r1,c)
                (planes at +RW). Step 3 is the odd tap (r2,c2) as a normal
                matmul (before step 4 so a late shifted copy never stalls
                it); step 4 pairs (r2,c0)+(r2,c1) via the col-shifted copy
                at +SHIFT and closes the accumulation group.
                """
                co = conv_idx * 1152
                ps3 = ps.rearrange("p (h w) -> p h w", w=RW)
                pout = ps3[:, 0:nrows, 0:W]
                if step < 3:
                    c = step
                    rhs = bass.AP(
                        tensor=src.tensor,
                        offset=src.offset + h0 * RW + c,
                        ap=[src.ap[0], [RW, 2], [RW, nrows], [1, W]],
                    )
                    lhsT = w_sb[:, co + c * 256 : co + (c + 1) * 256].rearrange(
                        "p (j m) -> p j m", j=2
                    )
                    nc.tensor.matmul(
                        pout, lhsT, rhs,
                        start=(step == STEPS[0]), stop=(step == STEPS[-1]),
                        perf_mode=DR, skip_group_check=True,
                    )
                elif step == 3:
                    rhs = bass.AP(
                        tensor=src.tensor,
                        offset=src.offset + (h0 + 2) * RW + 2,
                        ap=[src.ap[0], [RW, nrows], [1, W]],
                    )
                    nc.tensor.matmul(
                        pout, w_sb[:, co + 1024 : co + 1152], rhs,
                        start=(step == STEPS[0]), stop=(step == STEPS[-1]),
                        skip_group_check=True,
                    )
                else:
                    rhs = bass.AP(
                        tensor=src.tensor,
                        offset=src.offset + (h0 + 2) * RW,
                        ap=[src.ap[0], [SHIFT, 2], [RW, nrows], [1, W]],
                    )
                    lhsT = w_sb[:, co + 768 : co + 1024].rearrange(
                        "p (j m) -> p j m", j=2
                    )
                    nc.tensor.matmul(
                        pout, lhsT, rhs,
                        start=(step == STEPS[0]), stop=(step == STEPS[-1]),
                        perf_mode=DR, skip_group_check=True,
                    )

            def conv_chunk(ps, src, conv_idx, h0, nrows=CHUNK_ROWS):
                for step in STEPS:
                    chunk_matmul(ps, src, conv_idx, h0, step, nrows)

            def conv_chunk_pair(psA, psB, src, conv_idx, h0A, h0B,
                                nrowsA=CHUNK_ROWS, nrowsB=CHUNK_ROWS):
                """Two chunks with interleaved matmuls: alternating the two
                PSUM accumulation groups hides the per-group start/stop
                transition bubble on the PE."""
                for step in STEPS:
                    chunk_matmul(psA, src, conv_idx, h0A, step, nrowsA)
                    chunk_matmul(psB, src, conv_idx, h0B, step, nrowsB)

            # PE p-state warmup: the tensor clock ramps 0.65->2.4 GHz over
            # ~3us of continuous work, so burn the ramp on dummy matmuls
            # during the initial DMA window instead of on the real stream
            ps_warm = pspool.tile([C, NFLAT], F32, tag="ps1")
            lhsT_w = junk2[:, 0:256].rearrange("p (j m) -> p j m", j=2)
            for r in range(WARMUP_MM):
                rhs = bass.AP(
                    tensor=junk2.tensor, offset=junk2.offset,
                    ap=[junk2.ap[0], [64, 2], [1, 224]],
                )
                nc.tensor.matmul(
                    ps_warm[:, 0:224], lhsT_w, rhs,
                    start=(r == 0), stop=(r == WARMUP_MM - 1),
                    perf_mode=DR, skip_group_check=True,
                )

            # one-time pad prep: the ts rings need full pad zeroing; the xs
            # rings arrive with pads pre-baked from HBM and the on-chip
            # shift pieces cover every shifted row/col the matmuls read
            zero_pads(nc.vector, ts_t[0])
            zero_pads(nc.vector, ts_t[1])
            zero_pads(nc.vector, ts_t[2])

            def emit_inputs(i):
                """Input DMAs for image i >= 1, one whole-image transfer per
                ring (issued one image ahead): main sign image on sync, the
                residual on scalar. The shifted twin is built on-chip."""
                xs, xr = xs_t[i % 3], xr_t[i % 4]
                nc.sync.dma_start(xs[:, 0 : 58 * RW], s_d[i, :, 0 : 58 * RW])
                nc.scalar.dma_start(xr[:], xr_d[i])

            for i in range(BL):
                xs, ts = xs_t[i % 3], ts_t[i % 3]
                xr, o = xr_t[i % 4], o_t[i % 3]
                xs_n = xs_t[(i + 1) % 3]
                ts3 = xs3v(ts)
                o3 = o.rearrange("p h w -> p h w")
                if i + 1 < BL:
                    emit_inputs(i + 1)

                def evict1(ps1, h0):
                    # bn1 + sign (hardtanh folded into sign) -> conv2 input
                    ps1v = ps1.rearrange("p (h w) -> p h w", w=RW)[:, :, 0:W]
                    nc.scalar.activation(
                        ts3[:, 1 + h0 : 1 + h0 + CHUNK_ROWS, 1 : W + 1],
                        ps1v,
                        SIGN,
                        bias=bn_sb[:, 1:2],
                        scale=bn_sb[:, 0:1],
                    )
                    shift_scalar(ts, 1 + h0, CHUNK_ROWS)

                for k in range(0, N_CHUNKS - 1, 2):
                    if i == 0:
                        # image-0 shifted twin on the (otherwise idle) DVE,
                        # pieces racing just ahead of the chunk needing them
                        shift_copy(nc.vector, xs, SPIECES[k])
                        shift_copy(nc.vector, xs, SPIECES[k + 1])
                    h0A, h0B = k * CHUNK_ROWS, (k + 1) * CHUNK_ROWS
                    psA = pspool.tile([C, NFLAT], F32, tag="ps1")
                    psB = pspool.tile([C, NFLAT], F32, tag="ps1")
                    conv_chunk_pair(psA, psB, xs, 0, h0A, h0B)
                    evict1(psA, h0A)
                    evict1(psB, h0B)
                    if i == 0 and k == 0:
                        nc.scalar.dma_start(xr_t[0][:], xr_d[0])
                if i == 0:
                    shift_copy(nc.vector, xs, SPIECES[6])
                h0 = (N_CHUNKS - 1) * CHUNK_ROWS
                ps1 = pspool.tile([C, NFLAT], F32, tag="ps1")
                conv_chunk(ps1, xs, 0, h0)
                evict1(ps1, h0)
                if i + 1 < BL:
                    # image i+1's shifted twin on DVE while it is idle
                    # (its evict work only starts with conv2): the copies
                    # are ~0.4-1.1us each here vs 3.5-4.4us on GpSimd, and
                    # never queue in front of the PSUM-freeing affines
                    for p in SPIECES3:
                        shift_copy(nc.vector, xs_n, p)

                def evict2(ps2, k):
                    # out = clip(ps2*inv2 + (x+b2), -1, 1). The affine+add
                    # runs on DVE straight out of PSUM (frees the bank); the
                    # clip rides the GpSimd queue, in order before the y
                    # store that consumes it — this keeps DVE throughput
                    # per pair under the PE pair time so PSUM recycling
                    # never gates the matmul stream.
                    h0 = k * CHUNK_ROWS
                    ps2v = ps2.rearrange("p (h w) -> p h w", w=RW)[:, :, 0:W]
                    ov = o3[:, h0 : h0 + CHUNK_ROWS, :]
                    nc.vector.affine_then_add(
                        ov, ps2v, xr[:, h0 : h0 + CHUNK_ROWS, :],
                        scale=bn_sb[:, 2:3], bias=0.0,
                    )
                    nc.gpsimd.tensor_scalar(ov, ov, 1.0, -1.0, op0=MIN, op1=MAX)

                if i < BL - 1:
                    for k in range(0, N_CHUNKS - 1, 2):
                        psA = pspool.tile([C, NFLAT], F32, tag="ps2")
                        psB = pspool.tile([C, NFLAT], F32, tag="ps2")
                        conv_chunk_pair(
                            psA, psB, ts, 1, k * CHUNK_ROWS, (k + 1) * CHUNK_ROWS
                        )
                        evict2(psA, k)
                        evict2(psB, k + 1)
                    ps2 = pspool.tile([C, NFLAT], F32, tag="ps2")
                    conv_chunk(ps2, ts, 1, (N_CHUNKS - 1) * CHUNK_ROWS)
                    evict2(ps2, N_CHUNKS - 1)
                    # whole-image y store on the sync ring
                    nc.sync.dma_start(y_d[i], o3[:])
                else:
                    # Last image: conv1 is done, so both psum tags are free
                    # and there is no next image to overlap the tail with.
                    # y leaves in four pieces, each on the ring that is
                    # idle at that point, so the only post-stream work is
                    # the tiny 2-row store.
                    for kp in range(3):
                        kA, kB = 2 * kp, 2 * kp + 1
                        psA = pspool.tile([C, NFLAT], F32, tag="ps1")
                        psB = pspool.tile([C, NFLAT], F32, tag="ps2")
                        conv_chunk_pair(
                            psA, psB, ts, 1, kA * CHUNK_ROWS, kB * CHUNK_ROWS
                        )
                        evict2(psA, kA)
                        evict2(psB, kB)
                        # stagger the stores right behind each pair's clips
                        # so only the final 14 rows transfer after the last
                        # matmul (each [128,n] transfer costs ~1-2us of the
                        # shared DMA backend regardless of n)
                        if kp == 0:
                            nc.sync.dma_start(
                                y_d[i, :, 0:16, :], o3[:, 0:16, :]
                            )
                        elif kp == 1:
                            nc.sync.dma_start(
                                y_d[i, :, 16:32, :], o3[:, 16:32, :]
                            )
                        else:
                            nc.scalar.dma_start(
                                y_d[i, :, 32:40, :], o3[:, 32:40, :]
                            )
                            nc.sync.dma_start(
                                y_d[i, :, 40:48, :], o3[:, 40:48, :]
                            )

                    # final 6+2 pair: rows 48-54 affine on DVE + clip on
                    # GpSimd, rows 54-56 entirely on DVE — the two paths
                    # drain in parallel and each y piece rides its own
                    # ring so the last DMA posts soon after the final
                    # matmul.
                    h6 = (N_CHUNKS - 1) * CHUNK_ROWS
                    psA = pspool.tile([C, NFLAT], F32, tag="ps1")
                    psB = pspool.tile([C, NFLAT], F32, tag="ps2")
                    conv_chunk_pair(
                        psA, psB, ts, 1, h6, h6 + 6, nrowsA=6, nrowsB=2
                    )
                    psAv = psA.rearrange("p (h w) -> p h w", w=RW)[:, 0:6, 0:W]
                    ovA = o3[:, h6 : h6 + 6, :]
                    nc.vector.affine_then_add(
                        ovA, psAv, xr[:, h6 : h6 + 6, :],
                        scale=bn_sb[:, 2:3], bias=0.0,
                    )
                    nc.gpsimd.tensor_scalar(ovA, ovA, 1.0, -1.0, op0=MIN, op1=MAX)
                    nc.gpsimd.dma_start(
                        y_d[i, :, h6 : h6 + 6, :], o3[:, h6 : h6 + 6, :]
                    )
                    psBv = psB.rearrange("p (h w) -> p h w", w=RW)[:, 0:2, 0:W]
                    ovB = o3[:, h6 + 6 : h6 + 8, :]
                    nc.vector.affine_then_add(
                        ovB, psBv, xr[:, h6 + 6 : h6 + 8, :],
                        scale=bn_sb[:, 2:3], bias=0.0,
                    )
                    nc.vector.tensor_scalar(ovB, ovB, 1.0, -1.0, op0=MIN, op1=MAX)
                    nc.scalar.dma_start(
                        y_d[i, :, h6 + 6 : h6 + 8, :], o3[:, h6 + 6 : h6 + 8, :]
                    )

    nc.compile()
    return nc


def _get_nc():
    global _NC_CACHE
    if _NC_CACHE is None:
        _NC_CACHE = _build_nc()
    return _NC_CACHE


def kernel(
    x, w1, w2, gamma1, beta1, mean1, var1, gamma2, beta2, mean2, var2,
    trace=False,
):
    x = np.asarray(x, dtype=np.float32)
    w1 = np.asarray(w1, dtype=np.float32)
    w2 = np.asarray(w2, dtype=np.float32)

    # fold BN exactly as the reference does (f32 throughout)
    def fold(gamma, beta, mean, var):
        inv = (np.asarray(gamma, np.float32)
               / np.sqrt(np.asarray(var, np.float32) + np.float32(EPS)))
        b = np.asarray(beta, np.float32) - np.asarray(mean, np.float32) * inv
        return inv.astype(np.float32), b.astype(np.float32)

    inv1, b1 = fold(gamma1, beta1, mean1, var1)
    inv2, b2 = fold(gamma2, beta2, mean2, var2)
    bn_np = np.stack([inv1, b1, inv2, b2], axis=1).astype(np.float32)  # [C,4]

    # host prep: binarized input in the padded row-stride-64 layout (the
    # col-shifted twin half is filled on-chip), plus the b2-biased residual
    sg = np.sign(x).astype(ml_dtypes.float8_e4m3fn)
    sp = np.zeros((B, C, 2 * HP, RW), dtype=ml_dtypes.float8_e4m3fn)
    sp[:, :, 1 : H + 1, 1 : W + 1] = sg
    s_np = sp.reshape(B, C, 2 * SHIFT)
    xr_np = (x + b2[None, :, None, None]).astype(ml_dtypes.bfloat16)

    # fp8 weight tables; per conv: 3 DoubleRow pair tables, the (r2,c0)+
    # (r2,c1) pair, then the plain (r2,c2) table.
    # DR c=0..2: w_np[k, co + c*256 + j*128 + m] = sign(w[m,k,j,c]), j=row 0/1
    # DR #4:     pairs (r2,c0) j=0 and (r2,c1) j=1 at co+768
    # normal:    (r2,c2) at co+1024
    w_np = np.empty((C, WCOLS), dtype=ml_dtypes.float8_e4m3fn)
    for conv_idx, w in enumerate((w1, w2)):
        ws = np.sign(w).astype(ml_dtypes.float8_e4m3fn)  # [O, Cin, 3, 3]
        co = conv_idx * 1152
        for c in range(3):
            for j in range(2):
                w_np[:, co + c * 256 + j * 128 : co + c * 256 + (j + 1) * 128] = (
                    ws[:, :, j, c].T
                )
        w_np[:, co + 768 : co + 896] = ws[:, :, 2, 0].T
        w_np[:, co + 896 : co + 1024] = ws[:, :, 2, 1].T
        w_np[:, co + 1024 : co + 1152] = ws[:, :, 2, 2].T

    nc = _get_nc()
    in_maps = [
        {
            "s": s_np[i * BL : (i + 1) * BL],
            "xr": xr_np[i * BL : (i + 1) * BL],
            "w": w_np,
            "bn": bn_np,
        }
        for i in range(N_CORES)
    ]
    res = run_bass_kernel_spmd(
        nc, in_maps, core_ids=list(range(N_CORES)), trace=trace
    )
    y = np.concatenate(
        [np.asarray(res.results[i]["y"]) for i in range(N_CORES)], axis=0
    ).astype(np.float32)
    if trace:
        return y, res
    return y
